# revision 33
# baseline (speedup 1.0000x reference)
"""Trainium2 Bass kernel for a 6-layer post-BatchNorm transformer encoder.

Reference model:
  x = emb[seq] + pes                                  # [B,S,D] = [4,512,1024]
  6x: x = BN(x + attn(x)); x = BN(x + ffn(x))
  BN = per-channel batch stats over (B,S), eps=1e-3.

Sharding: data-parallel over tokens across 8 NeuronCores. Core c owns the
256 tokens  [batch c//2, sequence half c%2].  Weights are replicated
(streamed from HBM in bf16, host-converted).  Per layer the only
communication is:
  - a pair AllGather ([[0,1],[2,3],...]) exchanging K^T and token-major V
    (1MB bf16) so each core holds its batch's full 512-key sequence, and
  - two 8KB 8-rank AllReduces for the BatchNorm batch statistics
    (sum / sum-of-squares per channel).
This removes the TP-style [D,T] activation AllReduces entirely.

Numerics: matmul operands bf16 (weights + activation mirrors), PSUM
accumulation fp32, residual/BN arithmetic fp32.  x master kept fp32.
Attention per (head): scores^T = K_h @ Q_h^T (K=64 contraction),
E = exp(scale*scores^T) in bf16 (max-subtraction skipped; scores are O(1)),
U^T = V_h^T @ E^T with denominators from a ones-column appended to V,
normalized by a PE-broadcast reciprocal row.  bo/b2 biases cancel inside
BN and are dropped.  Host does input marshalling only: embedding lookup
(emb[seq]+pes), weight bf16 conversion + chunk-major relayout, output
reassembly.
"""

import os

import numpy as np

import concourse.bass as bass
import concourse.mybir as mybir
import concourse.tile as tile
from concourse import bacc
from concourse.bass import ts
from concourse.masks import make_identity

# ---------------------------------------------------------------- dims
V, D, L, H, B, S = 32000, 1024, 6, 16, 4, 512
HD = D // H            # 64
DF = 4 * D             # 4096
EPS = 1e-3
NC = 8                 # cores
T = B * S              # 2048 tokens (global, for BN stats)
P = 128                # partitions
TL = 256               # local tokens per core
DT = D // P            # 8 d-tiles
FT = DF // P           # 32 ffn hidden tiles
KT = S // P            # 4 key tiles (full sequence)

f32 = mybir.dt.float32
bf16 = mybir.dt.bfloat16
f32r = mybir.dt.float32r
AF = mybir.ActivationFunctionType
ALU = mybir.AluOpType

PAIRS = [[2 * i, 2 * i + 1] for i in range(4)]
ALL8 = [list(range(NC))]

N_LAYERS = int(os.environ.get("TRN_KERNEL_LAYERS", str(L)))
DEBUG_TAPS = os.environ.get("TRN_KERNEL_DEBUG", "0") == "1"


def _r(ap):
    """view an fp32 AP as float32r for full-rate PE matmul"""
    return ap.bitcast(f32r)


def build_module(n_layers=None):
    if n_layers is None:
        n_layers = N_LAYERS
    nc = bacc.Bacc("TRN2", target_bir_lowering=False, debug=False,
                   num_devices=NC)

    dt_ = nc.dram_tensor
    io = {
        "x0": dt_("x0", [P, DT, TL], f32, kind="ExternalInput").ap(),
        # weight chunks, host-prelaid so every chunk DMA is contiguous 1MB
        "wq": dt_("wq", [L, 2, P, DT, 512], bf16, kind="ExternalInput").ap(),
        "wk": dt_("wk", [L, 2, P, DT, 512], bf16, kind="ExternalInput").ap(),
        "wv": dt_("wv", [L, 2, P, DT, 512], bf16, kind="ExternalInput").ap(),
        "wo": dt_("wo", [L, 2, P, DT, 512], bf16, kind="ExternalInput").ap(),
        "w1": dt_("w1", [L, 8, P, DT, 512], bf16, kind="ExternalInput").ap(),
        "w2": dt_("w2", [L, 8, P, FT, P], bf16, kind="ExternalInput").ap(),
        "bq": dt_("bq", [L, P, DT], f32, kind="ExternalInput").ap(),
        "bk": dt_("bk", [L, P, DT], f32, kind="ExternalInput").ap(),
        "bv": dt_("bv", [L, P, DT], f32, kind="ExternalInput").ap(),
        "b1": dt_("b1", [L, P, FT], f32, kind="ExternalInput").ap(),
        "g1": dt_("g1", [L, P, DT], f32, kind="ExternalInput").ap(),
        "be1": dt_("be1", [L, P, DT], f32, kind="ExternalInput").ap(),
        "g2": dt_("g2", [L, P, DT], f32, kind="ExternalInput").ap(),
        "be2": dt_("be2", [L, P, DT], f32, kind="ExternalInput").ap(),
        "out": dt_("out", [D, TL], f32, kind="ExternalOutput").ap(),
    }
    if DEBUG_TAPS:
        for nm, shp, dt in [("dbg_q", [P, DT, TL], bf16),
                            ("dbg_k", [P, DT, S], bf16),
                            ("dbg_v", [P, KT, H, HD + 1], bf16),
                            ("dbg_attn", [P, DT, TL], bf16),
                            ("dbg_y1", [P, DT, TL], f32),
                            ("dbg_x2", [P, DT, TL], f32)]:
            io[nm] = dt_(nm, shp, dt, kind="ExternalOutput").ap()

    with tile.TileContext(nc) as tc:
        _build(tc, n_layers, io)
    nc.compile()
    return nc


def _build(tc, n_layers, io):
    from contextlib import ExitStack
    nc = tc.nc
    att_scale = 1.0 / np.sqrt(HD)

    # ------------------------------------------------ pools
    st = ExitStack()
    persist = st.enter_context(tc.tile_pool(name="persist", bufs=1))
    wc8 = st.enter_context(tc.tile_pool(name="wc8", bufs=5))    # [P,8,512] bf16
    wc32 = st.enter_context(tc.tile_pool(name="wc32", bufs=3))  # [P,32,128] bf16
    small = st.enter_context(tc.tile_pool(name="small", bufs=2))
    epool = st.enter_context(tc.tile_pool(name="epool", bufs=8))
    ps = st.enter_context(tc.tile_pool(name="ps", bufs=4, space="PSUM"))
    pu = st.enter_context(tc.tile_pool(name="pu", bufs=3, space="PSUM"))
    pst = st.enter_context(tc.tile_pool(name="pst", bufs=1, space="PSUM"))
    drin = st.enter_context(tc.tile_pool(name="drin", bufs=2, space="DRAM"))
    drout = st.enter_context(tc.tile_pool(name="drout", bufs=2, space="DRAM"))

    # ------------------------------------------------ persistent tiles
    x = persist.tile([P, DT, TL], f32, name="x")          # x master
    xb = persist.tile([P, DT, TL], bf16, name="xb")       # bf16 mirror
    y = persist.tile([P, DT, TL], f32, name="y")          # x + sublayer(x)
    qT = persist.tile([P, DT, TL], bf16, name="qT")
    kloc = persist.tile([P, DT, TL], bf16, name="kloc")   # local K^T
    vT = persist.tile([P, DT, TL], bf16, name="vT")       # local V^T
    vloc = persist.tile([P, 2, H, HD], bf16, name="vloc")  # local V tok-major
    kT = persist.tile([P, DT, S], bf16, name="kT")        # full K^T
    vsb = persist.tile([P, KT, H, HD + 1], bf16, name="vsb")  # V | ones
    attnT = persist.tile([P, DT, TL], bf16, name="attnT")
    h = persist.tile([P, FT, TL], bf16, name="h")         # ffn hidden
    identb = persist.tile([P, P], bf16, name="identb")

    make_identity(nc, identb[:])
    nc.vector.memset(vsb[:, :, :, HD:HD + 1], 1.0)        # ones lane

    # Dummy collective to absorb the one-time ~100us ncfw/driver warmup
    # while the embedding load and first projections run.
    wrm = persist.tile([P, 16], f32, name="wrm")
    nc.vector.memset(wrm[:], 0.0)
    wrin = drin.tile([P, 16], f32, tag="ari", name="wrin")
    wrout = drout.tile([P, 16], f32, tag="aro", addr_space="Shared",
                       name="wrout")
    nc.gpsimd.dma_start(wrin[:], wrm[:])
    nc.gpsimd.collective_compute(
        "AllReduce", ALU.add, replica_groups=ALL8,
        ins=[wrin.opt()], outs=[wrout.opt()])

    # ---------------------------------------- x = x0 (host: emb[seq]+pes)
    nc.sync.dma_start(x[:], io["x0"])
    for k in range(DT):
        nc.vector.tensor_copy(xb[:, k, :], x[:, k, :])

    # ---------------------------------------- batchnorm helpers
    def stats_partial(lbl, stt, mt):
        """channel sum + sumsq of y[:, mt, :] into stt columns."""
        nc.vector.reduce_sum(out=stt[:, mt:mt + 1], in_=y[:, mt, :],
                             axis=mybir.AxisListType.X)
        scr = epool.tile([P, TL], f32, tag="e", name=f"sq{lbl}_{mt}")
        nc.scalar.activation(scr[:], y[:, mt, :], AF.Square,
                             accum_out=stt[:, 8 + mt:9 + mt])

    def batchnorm(lbl, stt, g_sb, be_sb):
        """y -> x (fp32) and xb (bf16), exact global stats via 8KB AR."""
        arin = drin.tile([P, 16], f32, tag="ari", name=f"ari{lbl}")
        arout = drout.tile([P, 16], f32, tag="aro", addr_space="Shared",
                           name=f"aro{lbl}")
        nc.gpsimd.dma_start(arin[:], stt[:])
        nc.gpsimd.collective_compute(
            "AllReduce", ALU.add, replica_groups=ALL8,
            ins=[arin.opt()], outs=[arout.opt()])
        ast = small.tile([P, 16], f32, tag="ast", name=f"ast{lbl}")
        nc.gpsimd.dma_start(ast[:], arout[:])
        mean = small.tile([P, DT], f32, tag="mean", name=f"mean{lbl}")
        nc.vector.tensor_scalar_mul(mean[:], ast[:, 0:8], 1.0 / T)
        msq = small.tile([P, DT], f32, tag="msq", name=f"msq{lbl}")
        nc.vector.tensor_tensor(out=msq[:], in0=mean[:], in1=mean[:],
                                op=ALU.mult)
        veps = small.tile([P, DT], f32, tag="veps", name=f"veps{lbl}")
        nc.vector.scalar_tensor_tensor(out=veps[:], in0=ast[:, 8:16],
                                       scalar=1.0 / T, in1=msq[:],
                                       op0=ALU.mult, op1=ALU.subtract)
        nc.vector.tensor_scalar_add(veps[:], veps[:], EPS)
        rec = small.tile([P, DT], f32, tag="rec", name=f"rec{lbl}")
        nc.vector.reciprocal(rec[:], veps[:])
        rstd = small.tile([P, DT], f32, tag="rstd", name=f"rstd{lbl}")
        nc.scalar.sqrt(rstd[:], rec[:])
        sc = small.tile([P, DT], f32, tag="sc", name=f"sc{lbl}")
        nc.vector.tensor_tensor(out=sc[:], in0=g_sb[:], in1=rstd[:],
                                op=ALU.mult)
        sh = small.tile([P, DT], f32, tag="sh", name=f"sh{lbl}")
        nc.vector.tensor_tensor(out=sh[:], in0=mean[:], in1=sc[:], op=ALU.mult)
        nc.vector.tensor_tensor(out=sh[:], in0=be_sb[:], in1=sh[:],
                                op=ALU.subtract)
        for k in range(DT):
            nc.scalar.activation(x[:, k, :], y[:, k, :], AF.Identity,
                                 bias=sh[:, k:k + 1], scale=sc[:, k:k + 1])
            nc.vector.tensor_copy(xb[:, k, :], x[:, k, :])

    # ---------------------------------------- layers
    for l in range(n_layers):
        # ---- per-layer small params
        bq_sb = small.tile([P, DT], f32, tag="bq", name=f"bq{l}")
        bk_sb = small.tile([P, DT], f32, tag="bk", name=f"bk{l}")
        bv_sb = small.tile([P, DT], f32, tag="bv", name=f"bv{l}")
        b1_sb = small.tile([P, FT], f32, tag="b1", name=f"b1{l}")
        g1_sb = small.tile([P, DT], f32, tag="g1", name=f"g1{l}")
        be1_sb = small.tile([P, DT], f32, tag="be1", name=f"be1{l}")
        g2_sb = small.tile([P, DT], f32, tag="g2", name=f"g2{l}")
        be2_sb = small.tile([P, DT], f32, tag="be2", name=f"be2{l}")
        for nm, t_ in [("bq", bq_sb), ("bk", bk_sb), ("bv", bv_sb),
                       ("b1", b1_sb), ("g1", g1_sb), ("be1", be1_sb),
                       ("g2", g2_sb), ("be2", be2_sb)]:
            nc.sync.dma_start(t_[:], io[nm][l])

        # ---- QKV projections (local tokens, all heads); K first, V, then Q
        # so the K AllGather flies during V/Q compute, V's during Q.
        def proj(wname, dst, bias):
            for half in range(2):
                wcb = wc8.tile([P, DT, 512], bf16, tag="wc",
                               name=f"{wname}{l}_{half}")
                nc.sync.dma_start(wcb[:], io[wname][l, half])
                for m in range(4):
                    mt = half * 4 + m
                    psq = ps.tile([P, TL], f32, tag="mm",
                                  name=f"p{wname}{l}_{half}_{m}")
                    for k in range(DT):
                        nc.tensor.matmul(psq[:], wcb[:, k, ts(m, P)],
                                         xb[:, k, :],
                                         start=(k == 0), stop=(k == DT - 1))
                    nc.scalar.activation(dst[:, mt, :], psq[:], AF.Identity,
                                         bias=bias[:, mt:mt + 1])

        proj("wk", kloc, bk_sb)
        aginK = drin.tile([P, DT * TL], bf16, tag="agik", name=f"agik{l}")
        agoK = drout.tile([2, P, DT * TL], bf16, tag="agok", name=f"agok{l}")
        nc.gpsimd.dma_start(
            aginK[:].rearrange("p (k t) -> p k t", k=DT), kloc[:])
        nc.gpsimd.collective_compute(
            "AllGather", ALU.bypass, replica_groups=PAIRS,
            ins=[aginK.opt()], outs=[agoK.opt()])

        proj("wv", vT, bv_sb)
        # local V^T -> token-major V
        for tt in range(2):
            for k in range(DT):
                ptile = pst.tile([P, P], bf16, tag="tp", name=f"vt{l}_{tt}_{k}")
                nc.tensor.transpose(ptile[:], vT[:, k, ts(tt, P)], identb[:])
                nc.vector.tensor_copy(
                    vloc[:, tt, 2 * k:2 * k + 2, :],
                    ptile[:].rearrange("p (h e) -> p h e", h=2))
        aginV = drin.tile([P, DT * TL], bf16, tag="agiv", name=f"agiv{l}")
        agoV = drout.tile([2, P, DT * TL], bf16, tag="agov", name=f"agov{l}")
        nc.gpsimd.dma_start(
            aginV[:].rearrange("p (a h e) -> p a h e", a=2, h=H), vloc[:])
        nc.gpsimd.collective_compute(
            "AllGather", ALU.bypass, replica_groups=PAIRS,
            ins=[aginV.opt()], outs=[agoV.opt()])

        proj("wq", qT, bq_sb)

        for r in range(2):
            nc.gpsimd.dma_start(
                kT[:, :, r * TL:(r + 1) * TL],
                agoK[r].rearrange("p (k t) -> p k t", k=DT))
            nc.gpsimd.dma_start(
                vsb[:, 2 * r:2 * r + 2, :, 0:HD],
                agoV[r].rearrange("p (a h e) -> p a h e", a=2, h=H))

        # ---- attention, software-pipelined across head PAIRS; the even/odd
        # heads of a pair occupy disjoint PE row groups (partitions 0-63 vs
        # 64-127, K=64) so their score matmuls run concurrently.
        def pair_scores(p):
            ets = ([], [])
            for kt in range(KT):
                pp = []
                for sub in range(2):
                    pss = ps.tile([P, TL], f32, tag="mm",
                                  name=f"ps{l}_{p}_{sub}_{kt}")
                    hb = sub * HD
                    nc.tensor.matmul(pss[:], kT[hb:hb + HD, p, ts(kt, P)],
                                     qT[hb:hb + HD, p, :],
                                     start=True, stop=True)
                    pp.append(pss)
                for sub in range(2):
                    et = epool.tile([P, TL], bf16, tag="eb",
                                    name=f"et{l}_{p}_{sub}_{kt}")
                    nc.scalar.activation(et[:], pp[sub][:], AF.Exp,
                                         scale=att_scale)
                    ets[sub].append(et)
            return ets

        def pair_finish(p, ets):
            for sub in range(2):
                hd_ = 2 * p + sub
                hb = sub * HD
                psu = pu.tile([P, TL], f32, tag="u", name=f"pu{l}_{hd_}")
                for kt in range(KT):
                    nc.tensor.matmul(psu[0:HD + 1, :], vsb[:, kt, hd_, :],
                                     ets[sub][kt][:], start=(kt == 0),
                                     stop=(kt == KT - 1))
                rsb = epool.tile([1, TL], f32, tag="er", name=f"rs{l}_{hd_}")
                nc.vector.reciprocal(rsb[:], psu[HD:HD + 1, :])
                rbc = epool.tile([HD, TL], f32, tag="rb", name=f"rb{l}_{hd_}")
                nc.gpsimd.partition_broadcast(rbc[:], rsb[:])
                nc.vector.tensor_tensor(out=attnT[hb:hb + HD, p, :],
                                        in0=psu[0:HD, :], in1=rbc[:],
                                        op=ALU.mult)

        prev = None
        for p in range(H // 2):
            ets = pair_scores(p)
            if prev is not None:
                pair_finish(*prev)
            prev = (p, ets)
        pair_finish(*prev)

        # ---- Wo + residual -> y, stats partials inline
        stt1 = small.tile([P, 16], f32, tag="stt", name=f"stt_a{l}")
        for half in range(2):
            woc = wc8.tile([P, DT, 512], bf16, tag="wc", name=f"wo{l}_{half}")
            nc.sync.dma_start(woc[:], io["wo"][l, half])
            for m in range(4):
                mt = half * 4 + m
                ps2 = ps.tile([P, TL], f32, tag="mm", name=f"o{l}_{half}_{m}")
                for k in range(DT):
                    nc.tensor.matmul(ps2[:], woc[:, k, ts(m, P)],
                                     attnT[:, k, :],
                                     start=(k == 0), stop=(k == DT - 1))
                nc.vector.tensor_tensor(out=y[:, mt, :], in0=ps2[:],
                                        in1=x[:, mt, :], op=ALU.add)
                stats_partial(f"a{l}", stt1, mt)

        if DEBUG_TAPS and l == 0:
            nc.sync.dma_start(io["dbg_q"], qT[:])
            nc.sync.dma_start(io["dbg_k"], kT[:])
            nc.sync.dma_start(io["dbg_v"], vsb[:])
            nc.sync.dma_start(io["dbg_attn"], attnT[:])
            nc.sync.dma_start(io["dbg_y1"], y[:])

        # ---- BN1 -> x, xb
        batchnorm(f"a{l}", stt1, g1_sb, be1_sb)
        if DEBUG_TAPS and l == 0:
            nc.sync.dma_start(io["dbg_x2"], x[:])

        # ---- FFN1: h = relu(W1^T x + b1)
        for c in range(8):
            w1c = wc8.tile([P, DT, 512], bf16, tag="wc", name=f"w1{l}_{c}")
            nc.sync.dma_start(w1c[:], io["w1"][l, c])
            for m in range(4):
                mt = c * 4 + m
                ps1 = ps.tile([P, TL], f32, tag="mm", name=f"f{l}_{c}_{m}")
                for k in range(DT):
                    nc.tensor.matmul(ps1[:], w1c[:, k, ts(m, P)], xb[:, k, :],
                                     start=(k == 0), stop=(k == DT - 1))
                nc.scalar.activation(h[:, mt, :], ps1[:], AF.Relu,
                                     bias=b1_sb[:, mt:mt + 1])

        # ---- FFN2 + residual -> y, stats partials inline
        stt2 = small.tile([P, 16], f32, tag="stt", name=f"stt_f{l}")
        for m in range(DT):
            w2c = wc32.tile([P, FT, P], bf16, tag="wc2", name=f"w2{l}_{m}")
            nc.sync.dma_start(w2c[:], io["w2"][l, m])
            ps2 = pu.tile([P, TL], f32, tag="u", name=f"g{l}_{m}")
            for k in range(FT):
                nc.tensor.matmul(ps2[:], w2c[:, k, :], h[:, k, :],
                                 start=(k == 0), stop=(k == FT - 1))
            nc.vector.tensor_tensor(out=y[:, m, :], in0=ps2[:],
                                    in1=x[:, m, :], op=ALU.add)
            stats_partial(f"f{l}", stt2, m)

        # ---- BN2 -> x, xb
        batchnorm(f"f{l}", stt2, g2_sb, be2_sb)

    # ---------------------------------------- output x -> [D, TL]
    nc.sync.dma_start(io["out"].rearrange("(k p) t -> p k t", p=P), x[:])
    st.close()


# ================================================================ host side

def make_in_maps(inputs):
    import ml_dtypes
    f = lambda a: np.ascontiguousarray(np.asarray(a), dtype=np.float32)
    b = lambda a: np.ascontiguousarray(np.asarray(a, dtype=np.float32)
                                       .astype(ml_dtypes.bfloat16))
    seq = np.asarray(inputs["sequence"]).astype(np.int64)       # [B, S]
    emb = f(inputs["emb"])
    pes = f(inputs["pes"])
    x0 = emb[seq] + pes[None, :, :]                             # [B, S, D]

    Wq, Wk, Wv = inputs["Wq"], inputs["Wk"], inputs["Wv"]
    Wo, W1, W2 = inputs["Wo"], inputs["W1"], inputs["W2"]

    def chunk8(W, c):
        # [L, D, M] -> [L, c, P, DT, M//c]  with lhsT layout [k*P+p, m]
        W = np.asarray(W, dtype=np.float32)
        Lw, Dw, M = W.shape
        W = W.reshape(Lw, DT, P, c, M // c)
        return np.ascontiguousarray(
            W.transpose(0, 3, 2, 1, 4)).astype(ml_dtypes.bfloat16)

    def chunk_w2(W):
        # [L, DF, D] -> [L, 8, P, FT, P]: chunk m-tiles, k full
        W = np.asarray(W, dtype=np.float32)
        W = W.reshape(L, FT, P, DT, P)
        return np.ascontiguousarray(
            W.transpose(0, 3, 2, 1, 4)).astype(ml_dtypes.bfloat16)

    def vecP(v, n):
        # [L, n*P] -> [L, P, n]
        v = np.asarray(v, dtype=np.float32).reshape(L, n, P)
        return np.ascontiguousarray(v.transpose(0, 2, 1))

    shared = {
        "wq": chunk8(Wq, 2), "wk": chunk8(Wk, 2), "wv": chunk8(Wv, 2),
        "wo": chunk8(Wo, 2), "w1": chunk8(W1, 8), "w2": chunk_w2(W2),
        "bq": vecP(inputs["bq"], DT), "bk": vecP(inputs["bk"], DT),
        "bv": vecP(inputs["bv"], DT), "b1": vecP(inputs["b1"], FT),
        "g1": vecP(inputs["g1"], DT), "be1": vecP(inputs["be1"], DT),
        "g2": vecP(inputs["g2"], DT), "be2": vecP(inputs["be2"], DT),
    }

    in_maps = []
    for c in range(NC):
        bi, hf = c // 2, c % 2
        xs = x0[bi, hf * TL:(hf + 1) * TL, :]                   # [TL, D]
        x0T = np.ascontiguousarray(
            xs.T.reshape(DT, P, TL).transpose(1, 0, 2))          # [P, DT, TL]
        m = {"x0": x0T}
        m.update(shared)
        in_maps.append(m)
    return in_maps


def assemble_output(res):
    out = np.empty((B, S, D), dtype=np.float32)
    for c in range(NC):
        o = np.asarray(res.results[c]["out"])                   # [D, TL]
        bi, hf = c // 2, c % 2
        out[bi, hf * TL:(hf + 1) * TL, :] = o.T
    return out


_CACHE = {}


def _get_module():
    if "nc" not in _CACHE:
        _CACHE["nc"] = build_module()
    return _CACHE["nc"]


def kernel(**inputs):
    from concourse import bass_utils
    nc = _get_module()
    in_maps = make_in_maps(inputs)
    res = bass_utils.run_bass_kernel_spmd(nc, in_maps, list(range(NC)))
    return assemble_output(res)


# revision 36
# speedup vs baseline: 1.0635x; 1.0635x over previous
"""Trainium2 Bass kernel for a 6-layer post-BatchNorm transformer encoder.

Reference model:
  x = emb[seq] + pes                                  # [B,S,D] = [4,512,1024]
  6x: x = BN(x + attn(x)); x = BN(x + ffn(x))
  BN = per-channel batch stats over (B,S), eps=1e-3.

Sharding: data-parallel over tokens across 8 NeuronCores. Core c owns the
256 tokens  [batch c//2, sequence half c%2].  Weights are replicated
(streamed from HBM in bf16, host-converted).  Per layer the only
communication is:
  - a pair AllGather ([[0,1],[2,3],...]) exchanging K^T and token-major V
    (1MB bf16) so each core holds its batch's full 512-key sequence, and
  - two 8KB 8-rank AllReduces for the BatchNorm batch statistics
    (sum / sum-of-squares per channel).
This removes the TP-style [D,T] activation AllReduces entirely.

Numerics: matmul operands bf16 (weights + activation mirrors), PSUM
accumulation fp32, residual/BN arithmetic fp32.  x master kept fp32.
Attention per (head): scores^T = K_h @ Q_h^T (K=64 contraction),
E = exp(scale*scores^T) in bf16 (max-subtraction skipped; scores are O(1)),
U^T = V_h^T @ E^T with denominators from a ones-column appended to V,
normalized by a PE-broadcast reciprocal row.  bo/b2 biases cancel inside
BN and are dropped.  Host does input marshalling only: embedding lookup
(emb[seq]+pes), weight bf16 conversion + chunk-major relayout, output
reassembly.
"""

import os

import numpy as np

import concourse.bass as bass
import concourse.mybir as mybir
import concourse.tile as tile
from concourse import bacc
from concourse.bass import ts
from concourse.masks import make_identity

# ---------------------------------------------------------------- dims
V, D, L, H, B, S = 32000, 1024, 6, 16, 4, 512
HD = D // H            # 64
DF = 4 * D             # 4096
EPS = 1e-3
NC = 8                 # cores
T = B * S              # 2048 tokens (global, for BN stats)
P = 128                # partitions
TL = 256               # local tokens per core
DT = D // P            # 8 d-tiles
FT = DF // P           # 32 ffn hidden tiles
KT = S // P            # 4 key tiles (full sequence)

f32 = mybir.dt.float32
bf16 = mybir.dt.bfloat16
f32r = mybir.dt.float32r
AF = mybir.ActivationFunctionType
ALU = mybir.AluOpType

PAIRS = [[2 * i, 2 * i + 1] for i in range(4)]
ALL8 = [list(range(NC))]

N_LAYERS = int(os.environ.get("TRN_KERNEL_LAYERS", str(L)))
DEBUG_TAPS = os.environ.get("TRN_KERNEL_DEBUG", "0") == "1"


def _r(ap):
    """view an fp32 AP as float32r for full-rate PE matmul"""
    return ap.bitcast(f32r)


def build_module(n_layers=None):
    if n_layers is None:
        n_layers = N_LAYERS
    nc = bacc.Bacc("TRN2", target_bir_lowering=False, debug=False,
                   num_devices=NC)

    dt_ = nc.dram_tensor
    io = {
        "x0": dt_("x0", [P, DT, TL], f32, kind="ExternalInput").ap(),
        # weight chunks, host-prelaid so every chunk DMA is contiguous 1MB
        "wq": dt_("wq", [L, 2, P, DT, 512], bf16, kind="ExternalInput").ap(),
        "wk": dt_("wk", [L, 2, P, DT, 512], bf16, kind="ExternalInput").ap(),
        "wv": dt_("wv", [L, 2, P, DT, 512], bf16, kind="ExternalInput").ap(),
        "wo": dt_("wo", [L, 2, P, DT, 512], bf16, kind="ExternalInput").ap(),
        "w1": dt_("w1", [L, 8, P, DT, 512], bf16, kind="ExternalInput").ap(),
        "w2": dt_("w2", [L, 8, P, FT, P], bf16, kind="ExternalInput").ap(),
        "bq": dt_("bq", [L, P, DT], f32, kind="ExternalInput").ap(),
        "bk": dt_("bk", [L, P, DT], f32, kind="ExternalInput").ap(),
        "bv": dt_("bv", [L, P, DT], f32, kind="ExternalInput").ap(),
        "b1": dt_("b1", [L, P, FT], f32, kind="ExternalInput").ap(),
        "g1": dt_("g1", [L, P, DT], f32, kind="ExternalInput").ap(),
        "be1": dt_("be1", [L, P, DT], f32, kind="ExternalInput").ap(),
        "g2": dt_("g2", [L, P, DT], f32, kind="ExternalInput").ap(),
        "be2": dt_("be2", [L, P, DT], f32, kind="ExternalInput").ap(),
        "out": dt_("out", [D, TL], f32, kind="ExternalOutput").ap(),
    }
    if DEBUG_TAPS:
        for nm, shp, dt in [("dbg_q", [P, DT, TL], bf16),
                            ("dbg_k", [P, DT, S], bf16),
                            ("dbg_v", [P, KT, H, HD + 1], bf16),
                            ("dbg_attn", [P, DT, TL], bf16),
                            ("dbg_y1", [P, DT, TL], f32),
                            ("dbg_x2", [P, DT, TL], f32)]:
            io[nm] = dt_(nm, shp, dt, kind="ExternalOutput").ap()

    with tile.TileContext(nc) as tc:
        _build(tc, n_layers, io)
    nc.compile()
    return nc


def _build(tc, n_layers, io):
    from contextlib import ExitStack
    nc = tc.nc
    att_scale = 1.0 / np.sqrt(HD)

    # ------------------------------------------------ pools
    st = ExitStack()
    persist = st.enter_context(tc.tile_pool(name="persist", bufs=1))
    wc8 = st.enter_context(tc.tile_pool(name="wc8", bufs=5))    # [P,8,512] bf16
    wc32 = st.enter_context(tc.tile_pool(name="wc32", bufs=3))  # [P,32,128] bf16
    small = st.enter_context(tc.tile_pool(name="small", bufs=2))
    epool = st.enter_context(tc.tile_pool(name="epool", bufs=8))
    ps = st.enter_context(tc.tile_pool(name="ps", bufs=4, space="PSUM"))
    pu = st.enter_context(tc.tile_pool(name="pu", bufs=3, space="PSUM"))
    pst = st.enter_context(tc.tile_pool(name="pst", bufs=1, space="PSUM"))
    drin = st.enter_context(tc.tile_pool(name="drin", bufs=2, space="DRAM"))
    drout = st.enter_context(tc.tile_pool(name="drout", bufs=2, space="DRAM"))

    # ------------------------------------------------ persistent tiles
    x = persist.tile([P, DT, TL], f32, name="x")          # x master
    xb = persist.tile([P, DT, TL], bf16, name="xb")       # bf16 mirror
    y = persist.tile([P, DT, TL], f32, name="y")          # x + sublayer(x)
    qT = persist.tile([P, DT, TL], bf16, name="qT")
    kloc = persist.tile([P, DT, TL], bf16, name="kloc")   # local K^T
    vT = persist.tile([P, DT, TL], bf16, name="vT")       # local V^T
    vloc = persist.tile([P, 2, H, HD], bf16, name="vloc")  # local V tok-major
    kT = persist.tile([P, DT, S], bf16, name="kT")        # full K^T
    vsb = persist.tile([P, KT, H, HD + 1], bf16, name="vsb")  # V | ones
    attnT = persist.tile([P, DT, TL], bf16, name="attnT")
    h = persist.tile([P, FT, TL], bf16, name="h")         # ffn hidden
    identb = persist.tile([P, P], bf16, name="identb")

    make_identity(nc, identb[:])
    nc.vector.memset(vsb[:, :, :, HD:HD + 1], 1.0)        # ones lane

    # Dummy collective to absorb the one-time ~100us ncfw/driver warmup
    # while the embedding load and first projections run.
    wrm = persist.tile([P, 16], f32, name="wrm")
    nc.vector.memset(wrm[:], 0.0)
    wrin = drin.tile([P, 16], f32, tag="ari", name="wrin")
    wrout = drout.tile([P, 16], f32, tag="aro", addr_space="Shared",
                       name="wrout")
    nc.gpsimd.dma_start(wrin[:], wrm[:])
    nc.gpsimd.collective_compute(
        "AllReduce", ALU.add, replica_groups=ALL8,
        ins=[wrin.opt()], outs=[wrout.opt()])

    # ---------------------------------------- x = x0 (host: emb[seq]+pes)
    nc.sync.dma_start(x[:], io["x0"])
    for k in range(DT):
        nc.vector.tensor_copy(xb[:, k, :], x[:, k, :])

    # ---------------------------------------- batchnorm helpers
    def stats_partial(lbl, stt, mt):
        """channel sum + sumsq of y[:, mt, :] into stt columns."""
        nc.vector.reduce_sum(out=stt[:, mt:mt + 1], in_=y[:, mt, :],
                             axis=mybir.AxisListType.X)
        scr = epool.tile([P, TL], f32, tag="e", name=f"sq{lbl}_{mt}")
        nc.scalar.activation(scr[:], y[:, mt, :], AF.Square,
                             accum_out=stt[:, 8 + mt:9 + mt])

    def batchnorm(lbl, stt, g_sb, be_sb):
        """y -> x (fp32) and xb (bf16), exact global stats via 8KB AR."""
        arin = drin.tile([P, 16], f32, tag="ari", name=f"ari{lbl}")
        arout = drout.tile([P, 16], f32, tag="aro", addr_space="Shared",
                           name=f"aro{lbl}")
        nc.gpsimd.dma_start(arin[:], stt[:])
        nc.gpsimd.collective_compute(
            "AllReduce", ALU.add, replica_groups=ALL8,
            ins=[arin.opt()], outs=[arout.opt()])
        ast = small.tile([P, 16], f32, tag="ast", name=f"ast{lbl}")
        nc.gpsimd.dma_start(ast[:], arout[:])
        mean = small.tile([P, DT], f32, tag="mean", name=f"mean{lbl}")
        nc.vector.tensor_scalar_mul(mean[:], ast[:, 0:8], 1.0 / T)
        msq = small.tile([P, DT], f32, tag="msq", name=f"msq{lbl}")
        nc.vector.tensor_tensor(out=msq[:], in0=mean[:], in1=mean[:],
                                op=ALU.mult)
        veps = small.tile([P, DT], f32, tag="veps", name=f"veps{lbl}")
        nc.vector.scalar_tensor_tensor(out=veps[:], in0=ast[:, 8:16],
                                       scalar=1.0 / T, in1=msq[:],
                                       op0=ALU.mult, op1=ALU.subtract)
        nc.vector.tensor_scalar_add(veps[:], veps[:], EPS)
        rec = small.tile([P, DT], f32, tag="rec", name=f"rec{lbl}")
        nc.vector.reciprocal_approx_fast(rec[:], veps[:])
        rstd = small.tile([P, DT], f32, tag="rstd", name=f"rstd{lbl}")
        nc.scalar.sqrt(rstd[:], rec[:])
        sc = small.tile([P, DT], f32, tag="sc", name=f"sc{lbl}")
        nc.vector.tensor_tensor(out=sc[:], in0=g_sb[:], in1=rstd[:],
                                op=ALU.mult)
        sh = small.tile([P, DT], f32, tag="sh", name=f"sh{lbl}")
        nc.vector.tensor_tensor(out=sh[:], in0=mean[:], in1=sc[:], op=ALU.mult)
        nc.vector.tensor_tensor(out=sh[:], in0=be_sb[:], in1=sh[:],
                                op=ALU.subtract)
        for k in range(DT):
            nc.scalar.activation(x[:, k, :], y[:, k, :], AF.Identity,
                                 bias=sh[:, k:k + 1], scale=sc[:, k:k + 1])
            nc.vector.tensor_copy(xb[:, k, :], x[:, k, :])

    # ---------------------------------------- layers
    for l in range(n_layers):
        # ---- per-layer small params
        bq_sb = small.tile([P, DT], f32, tag="bq", name=f"bq{l}")
        bk_sb = small.tile([P, DT], f32, tag="bk", name=f"bk{l}")
        bv_sb = small.tile([P, DT], f32, tag="bv", name=f"bv{l}")
        b1_sb = small.tile([P, FT], f32, tag="b1", name=f"b1{l}")
        g1_sb = small.tile([P, DT], f32, tag="g1", name=f"g1{l}")
        be1_sb = small.tile([P, DT], f32, tag="be1", name=f"be1{l}")
        g2_sb = small.tile([P, DT], f32, tag="g2", name=f"g2{l}")
        be2_sb = small.tile([P, DT], f32, tag="be2", name=f"be2{l}")
        for nm, t_ in [("bq", bq_sb), ("bk", bk_sb), ("bv", bv_sb),
                       ("b1", b1_sb), ("g1", g1_sb), ("be1", be1_sb),
                       ("g2", g2_sb), ("be2", be2_sb)]:
            nc.sync.dma_start(t_[:], io[nm][l])

        # ---- QKV projections (local tokens, all heads); K first, V, then Q
        # so the K AllGather flies during V/Q compute, V's during Q.
        def proj(wname, dst, bias):
            for half in range(2):
                wcb = wc8.tile([P, DT, 512], bf16, tag="wc",
                               name=f"{wname}{l}_{half}")
                nc.sync.dma_start(wcb[:], io[wname][l, half])
                for m in range(4):
                    mt = half * 4 + m
                    psq = ps.tile([P, TL], f32, tag="mm",
                                  name=f"p{wname}{l}_{half}_{m}")
                    for k in range(DT):
                        nc.tensor.matmul(psq[:], wcb[:, k, ts(m, P)],
                                         xb[:, k, :],
                                         start=(k == 0), stop=(k == DT - 1))
                    nc.scalar.activation(dst[:, mt, :], psq[:], AF.Identity,
                                         bias=bias[:, mt:mt + 1])

        proj("wk", kloc, bk_sb)
        aginK = drin.tile([P, DT * TL], bf16, tag="agik", name=f"agik{l}")
        agoK = drout.tile([2, P, DT * TL], bf16, tag="agok", name=f"agok{l}")
        nc.gpsimd.dma_start(
            aginK[:].rearrange("p (k t) -> p k t", k=DT), kloc[:])
        nc.gpsimd.collective_compute(
            "AllGather", ALU.bypass, replica_groups=PAIRS,
            ins=[aginK.opt()], outs=[agoK.opt()])

        proj("wv", vT, bv_sb)
        # local V^T -> token-major V
        for tt in range(2):
            for k in range(DT):
                ptile = pst.tile([P, P], bf16, tag="tp", name=f"vt{l}_{tt}_{k}")
                nc.tensor.transpose(ptile[:], vT[:, k, ts(tt, P)], identb[:])
                nc.vector.tensor_copy(
                    vloc[:, tt, 2 * k:2 * k + 2, :],
                    ptile[:].rearrange("p (h e) -> p h e", h=2))
        aginV = drin.tile([P, DT * TL], bf16, tag="agiv", name=f"agiv{l}")
        agoV = drout.tile([2, P, DT * TL], bf16, tag="agov", name=f"agov{l}")
        nc.gpsimd.dma_start(
            aginV[:].rearrange("p (a h e) -> p a h e", a=2, h=H), vloc[:])
        nc.gpsimd.collective_compute(
            "AllGather", ALU.bypass, replica_groups=PAIRS,
            ins=[aginV.opt()], outs=[agoV.opt()])

        proj("wq", qT, bq_sb)

        for r in range(2):
            nc.gpsimd.dma_start(
                kT[:, :, r * TL:(r + 1) * TL],
                agoK[r].rearrange("p (k t) -> p k t", k=DT))
            nc.gpsimd.dma_start(
                vsb[:, 2 * r:2 * r + 2, :, 0:HD],
                agoV[r].rearrange("p (a h e) -> p a h e", a=2, h=H))

        # ---- attention, software-pipelined across head PAIRS; the even/odd
        # heads of a pair occupy disjoint PE row groups (partitions 0-63 vs
        # 64-127, K=64) so their score matmuls run concurrently.
        def pair_scores(p):
            ets = ([], [])
            for kt in range(KT):
                pp = []
                for sub in range(2):
                    pss = ps.tile([P, TL], f32, tag="mm",
                                  name=f"ps{l}_{p}_{sub}_{kt}")
                    hb = sub * HD
                    nc.tensor.matmul(pss[:], kT[hb:hb + HD, p, ts(kt, P)],
                                     qT[hb:hb + HD, p, :],
                                     start=True, stop=True)
                    pp.append(pss)
                for sub in range(2):
                    et = epool.tile([P, TL], bf16, tag="eb",
                                    name=f"et{l}_{p}_{sub}_{kt}")
                    nc.scalar.activation(et[:], pp[sub][:], AF.Exp,
                                         scale=att_scale)
                    ets[sub].append(et)
            return ets

        def pair_finish(p, ets):
            for sub in range(2):
                hd_ = 2 * p + sub
                hb = sub * HD
                psu = pu.tile([P, TL], f32, tag="u", name=f"pu{l}_{hd_}")
                for kt in range(KT):
                    nc.tensor.matmul(psu[0:HD + 1, :], vsb[:, kt, hd_, :],
                                     ets[sub][kt][:], start=(kt == 0),
                                     stop=(kt == KT - 1))
                rsb = epool.tile([1, TL], f32, tag="er", name=f"rs{l}_{hd_}")
                nc.vector.reciprocal(rsb[:], psu[HD:HD + 1, :])
                rbc = epool.tile([HD, TL], f32, tag="rb", name=f"rb{l}_{hd_}")
                nc.gpsimd.partition_broadcast(rbc[:], rsb[:])
                nc.vector.tensor_tensor(out=attnT[hb:hb + HD, p, :],
                                        in0=psu[0:HD, :], in1=rbc[:],
                                        op=ALU.mult)

        prev = None
        for p in range(H // 2):
            ets = pair_scores(p)
            if prev is not None:
                pair_finish(*prev)
            prev = (p, ets)
        pair_finish(*prev)

        # ---- Wo + residual -> y, stats partials inline
        stt1 = small.tile([P, 16], f32, tag="stt", name=f"stt_a{l}")
        for half in range(2):
            woc = wc8.tile([P, DT, 512], bf16, tag="wc", name=f"wo{l}_{half}")
            nc.sync.dma_start(woc[:], io["wo"][l, half])
            for m in range(4):
                mt = half * 4 + m
                ps2 = ps.tile([P, TL], f32, tag="mm", name=f"o{l}_{half}_{m}")
                for k in range(DT):
                    nc.tensor.matmul(ps2[:], woc[:, k, ts(m, P)],
                                     attnT[:, k, :],
                                     start=(k == 0), stop=(k == DT - 1))
                nc.vector.tensor_tensor(out=y[:, mt, :], in0=ps2[:],
                                        in1=x[:, mt, :], op=ALU.add)
                stats_partial(f"a{l}", stt1, mt)

        if DEBUG_TAPS and l == 0:
            nc.sync.dma_start(io["dbg_q"], qT[:])
            nc.sync.dma_start(io["dbg_k"], kT[:])
            nc.sync.dma_start(io["dbg_v"], vsb[:])
            nc.sync.dma_start(io["dbg_attn"], attnT[:])
            nc.sync.dma_start(io["dbg_y1"], y[:])

        # ---- BN1 -> x, xb
        batchnorm(f"a{l}", stt1, g1_sb, be1_sb)
        if DEBUG_TAPS and l == 0:
            nc.sync.dma_start(io["dbg_x2"], x[:])

        # ---- FFN1: h = relu(W1^T x + b1)
        for c in range(8):
            w1c = wc8.tile([P, DT, 512], bf16, tag="wc", name=f"w1{l}_{c}")
            nc.sync.dma_start(w1c[:], io["w1"][l, c])
            for m in range(4):
                mt = c * 4 + m
                ps1 = ps.tile([P, TL], f32, tag="mm", name=f"f{l}_{c}_{m}")
                for k in range(DT):
                    nc.tensor.matmul(ps1[:], w1c[:, k, ts(m, P)], xb[:, k, :],
                                     start=(k == 0), stop=(k == DT - 1))
                nc.scalar.activation(h[:, mt, :], ps1[:], AF.Relu,
                                     bias=b1_sb[:, mt:mt + 1])

        # ---- FFN2 + residual -> y, stats partials inline
        stt2 = small.tile([P, 16], f32, tag="stt", name=f"stt_f{l}")
        for m in range(DT):
            w2c = wc32.tile([P, FT, P], bf16, tag="wc2", name=f"w2{l}_{m}")
            nc.sync.dma_start(w2c[:], io["w2"][l, m])
            ps2 = pu.tile([P, TL], f32, tag="u", name=f"g{l}_{m}")
            for k in range(FT):
                nc.tensor.matmul(ps2[:], w2c[:, k, :], h[:, k, :],
                                 start=(k == 0), stop=(k == FT - 1))
            nc.vector.tensor_tensor(out=y[:, m, :], in0=ps2[:],
                                    in1=x[:, m, :], op=ALU.add)
            stats_partial(f"f{l}", stt2, m)

        # ---- BN2 -> x, xb
        batchnorm(f"f{l}", stt2, g2_sb, be2_sb)

    # ---------------------------------------- output x -> [D, TL]
    nc.sync.dma_start(io["out"].rearrange("(k p) t -> p k t", p=P), x[:])
    st.close()


# ================================================================ host side

def make_in_maps(inputs):
    import ml_dtypes
    f = lambda a: np.ascontiguousarray(np.asarray(a), dtype=np.float32)
    b = lambda a: np.ascontiguousarray(np.asarray(a, dtype=np.float32)
                                       .astype(ml_dtypes.bfloat16))
    seq = np.asarray(inputs["sequence"]).astype(np.int64)       # [B, S]
    emb = f(inputs["emb"])
    pes = f(inputs["pes"])
    x0 = emb[seq] + pes[None, :, :]                             # [B, S, D]

    Wq, Wk, Wv = inputs["Wq"], inputs["Wk"], inputs["Wv"]
    Wo, W1, W2 = inputs["Wo"], inputs["W1"], inputs["W2"]

    def chunk8(W, c):
        # [L, D, M] -> [L, c, P, DT, M//c]  with lhsT layout [k*P+p, m]
        W = np.asarray(W, dtype=np.float32)
        Lw, Dw, M = W.shape
        W = W.reshape(Lw, DT, P, c, M // c)
        return np.ascontiguousarray(
            W.transpose(0, 3, 2, 1, 4)).astype(ml_dtypes.bfloat16)

    def chunk_w2(W):
        # [L, DF, D] -> [L, 8, P, FT, P]: chunk m-tiles, k full
        W = np.asarray(W, dtype=np.float32)
        W = W.reshape(L, FT, P, DT, P)
        return np.ascontiguousarray(
            W.transpose(0, 3, 2, 1, 4)).astype(ml_dtypes.bfloat16)

    def vecP(v, n):
        # [L, n*P] -> [L, P, n]
        v = np.asarray(v, dtype=np.float32).reshape(L, n, P)
        return np.ascontiguousarray(v.transpose(0, 2, 1))

    shared = {
        "wq": chunk8(Wq, 2), "wk": chunk8(Wk, 2), "wv": chunk8(Wv, 2),
        "wo": chunk8(Wo, 2), "w1": chunk8(W1, 8), "w2": chunk_w2(W2),
        "bq": vecP(inputs["bq"], DT), "bk": vecP(inputs["bk"], DT),
        "bv": vecP(inputs["bv"], DT), "b1": vecP(inputs["b1"], FT),
        "g1": vecP(inputs["g1"], DT), "be1": vecP(inputs["be1"], DT),
        "g2": vecP(inputs["g2"], DT), "be2": vecP(inputs["be2"], DT),
    }

    in_maps = []
    for c in range(NC):
        bi, hf = c // 2, c % 2
        xs = x0[bi, hf * TL:(hf + 1) * TL, :]                   # [TL, D]
        x0T = np.ascontiguousarray(
            xs.T.reshape(DT, P, TL).transpose(1, 0, 2))          # [P, DT, TL]
        m = {"x0": x0T}
        m.update(shared)
        in_maps.append(m)
    return in_maps


def assemble_output(res):
    out = np.empty((B, S, D), dtype=np.float32)
    for c in range(NC):
        o = np.asarray(res.results[c]["out"])                   # [D, TL]
        bi, hf = c // 2, c % 2
        out[bi, hf * TL:(hf + 1) * TL, :] = o.T
    return out


_CACHE = {}


def _get_module():
    if "nc" not in _CACHE:
        _CACHE["nc"] = build_module()
    return _CACHE["nc"]


def kernel(**inputs):
    from concourse import bass_utils
    nc = _get_module()
    in_maps = make_in_maps(inputs)
    res = bass_utils.run_bass_kernel_spmd(nc, in_maps, list(range(NC)))
    return assemble_output(res)
